# revision 21
# baseline (speedup 1.0000x reference)
"""FP8 GEMM kernel (MixLinear) for 8 trn2 NeuronCores.

Reference computation:
    s      = max(|x|) / 448                        (global fp32 scalar)
    q_x    = e4m3fn(clip(x / s, +-448))            (OCP e4m3fn)
    q_w    = e4m3fn(clip(w, +-448))                (scale_weight = 1)
    y      = (q_x @ q_w.T) * s + bias              (fp32 accum -> fp16)

Strategy: data-parallel over the 16384 token rows (2048 rows per core).
Host does layout only; device does amax, a cross-core max-exchange,
quantization, DoubleRow fp8 matmul and scale+bias eviction.

TRN e4m3 tops out at 240 (vs OCP 448), so x is quantized at half scale:
    q_half = trn_e4m3(x * (224/gmax))  ==  ocp_e4m3(x / s) / 2
exactly for all magnitudes >= 2^-6 * s; weights (|w| <= 1/sqrt(2048))
are in the range where the TRN and OCP grids agree exactly, so they are
quantized at scale 1.  The output scale is then 2*s = gmax/224.

Schedule (evolved over seven traced iterations):
  - Global max via ONE remote_dma_broadcast all-to-all (hardware SDMA
    + semaphores): every core sends its per-partition |x|-max vector
    to column `me` of every core's inbox, waits 16 remote-sem incs
    (2 per sender), max-reduces [128,8].  This avoids the CC-engine
    AllGather (ncfw barrier + arming => scale ready 90us+).
  - Critical-section entry normally waits a global-clock snapshot of
    everything emitted before it, so the exchange section is emitted
    BEFORE the weight loop (snapshot = x load + amax only) and uses
    tc.wait_critical_data_deps() so the Pool engine runs the SWDGE
    descgen during the amax and only the trigger waits for the fold.
  - The SWDGE ucode library is prewarmed by a self-directed
    remote_sem_update in a first critical section (~6us load, runs
    ~8-14us under the x DMA); gpsimd runs no other custom-lib ops so
    the library stays resident.  partition_id is pre-cached.
  - A trailing dummy CC AllGather keeps a collective in the NEFF:
    without one the runtime launches the 8 cores unsynchronized
    (multi-ms skew observed); it completes under the matmul phase.
  - x tiles then weights are nosync-dep chained on one DMA ring:
    ordering without serialization (sync-chaining measured 124 GB/s).
  - amax: full-tile DVE reduces (first two tiles halved to start the
    pipeline earlier); DVE sustains ~119 G elem/s which makes it the
    prefix binder; GPSIMD cross-lane reduce is slower and would evict
    the SWDGE library.
  - bias broadcast via PE ones-outer-product with ACT PSUM->SBUF
    copies (ACT is idle early; keeps DVE clear for the fold).
  - Matmul j-outer over the first mt-pair only (consumes weight casts
    as ACT produces them), nt-outer/j-inner with per-bank eviction
    pipelining afterwards; quantization emitted in consumption order:
    pair0 on DVE, mt2-3 on ACT (frees DVE for pair0 evictions), then
    split DVE even-j / ACT odd-j.
"""

import numpy as np

B, S, D_IN, D_OUT = 2, 8192, 2048, 2048
N_CORES = 8
TOK = B * S                  # 16384
TOK_PC = TOK // N_CORES      # 2048 token rows per core
P = 128
KP = D_IN // (2 * P)         # 8 k-pairs of 256 (DoubleRow granularity)
MT = TOK_PC // P             # 16 token tiles per core
N_TILE = 512
NT = D_OUT // N_TILE         # 4 output column tiles

_compiled = None


def _build():
    import concourse.bacc as bacc
    import concourse.tile as tile
    from concourse import mybir
    from concourse.bass import DynSlice, _add_dep_helper
    from concourse.masks import make_identity

    f16 = mybir.dt.float16
    f32 = mybir.dt.float32
    f8 = mybir.dt.float8e4
    Alu = mybir.AluOpType
    Axis = mybir.AxisListType
    Act = mybir.ActivationFunctionType

    nc = bacc.Bacc("TRN2", target_bir_lowering=False, debug=False,
                   num_devices=N_CORES)

    # xt: x^T shard [d_in, tok_pc]; wt: w^T [d_in, d_out] (replicated)
    xt = nc.dram_tensor("xt", [D_IN, TOK_PC], f16, kind="ExternalInput")
    wt = nc.dram_tensor("wt", [D_IN, D_OUT], f16, kind="ExternalInput")
    bias = nc.dram_tensor("bias", [D_OUT], f16, kind="ExternalInput")
    y = nc.dram_tensor("y", [TOK_PC, D_OUT], f16, kind="ExternalOutput")

    # bounce buffers for the trailing launch-sync AllGather
    cc_in = nc.dram_tensor("cc_in", [16], f32)
    cc_out = nc.dram_tensor("cc_out", [16 * N_CORES], f32, addr_space="Shared")
    groups = [list(range(N_CORES))]

    def order(inst, prev, why):
        if prev is not None:
            _add_dep_helper(inst.ins, prev.ins, sync=False, reason=why)
        return inst

    with tile.TileContext(nc) as tc:
        with (
            tc.tile_pool(name="xpool", bufs=KP) as xpool,
            tc.tile_pool(name="qxpool", bufs=KP) as qxpool,
            tc.tile_pool(name="qwpool", bufs=KP) as qwpool,
            tc.tile_pool(name="wstage", bufs=3) as wstage,
            tc.tile_pool(name="small", bufs=1) as small,
            tc.tile_pool(name="ypool", bufs=8) as ypool,
            tc.tile_pool(name="psum", bufs=8, space="PSUM") as psum,
        ):
            # cache partition id on gpsimd (reg loads, off critical path)
            nc.gpsimd._cached_partition_id = nc.gpsimd.partition_id()

            # ---- SWDGE ucode prewarm: self-directed sem-only update in
            # an early critical section; loads the library (~6us) under
            # the x DMA and nothing evicts it afterwards ----
            junk = nc.alloc_semaphore("xmax_junk")
            lsem = nc.alloc_semaphore("xmax_lsem")
            psem = nc.alloc_semaphore("xmax_psem")
            with tc.tile_critical(name="warm"):
                rd0: list = [None] * 8
                rd0[0] = (0, 0)
                nc.gpsimd.remote_sem_update_broadcast(
                    junk, lsem, rdests=rd0).then_inc(psem, 1)
                nc.gpsimd.wait_ge(psem, 1)
                nc.gpsimd.trigger_dma(count=1)

            # ---- Phase A: x load (nosync-ordered ring), DVE abs-max
            # reduce per tile as it lands (first two tiles halved) ----
            pmax = small.tile([P, KP + 2], f32)
            nc.vector.memset(pmax[:], 0.0)
            x_sb = []
            prev_dma = None
            HT = TOK_PC // 2
            for j in range(KP):
                t = xpool.tile([P, 2, TOK_PC], f16, tag="xsb")
                src = xt[2 * j * P:(2 * j + 2) * P, :]
                src = src.rearrange("(p t) m -> p t m", t=2)
                if j < 2:
                    for h in range(2):
                        sl = slice(0, HT) if h == 0 else slice(HT, TOK_PC)
                        dma = nc.sync.dma_start(t[:, :, sl], src[:, :, sl])
                        prev_dma = order(dma, prev_dma, "x-order")
                        nc.vector.tensor_reduce(
                            out=pmax[:, 2 * j + h:2 * j + h + 1],
                            in_=t[:, :, sl], axis=Axis.XY,
                            op=Alu.max, apply_absolute_value=True)
                else:
                    dma = nc.sync.dma_start(t[:], src)
                    prev_dma = order(dma, prev_dma, "x-order")
                    nc.vector.tensor_reduce(
                        out=pmax[:, j + 2:j + 3], in_=t[:], axis=Axis.XY,
                        op=Alu.max, apply_absolute_value=True)
                x_sb.append(t)

            # ---- fold local partials into one [128,1] vector ----
            cur0 = small.tile([P, 1], f32, name="cur0")
            nc.vector.tensor_reduce(out=cur0[:], in_=pmax[:], axis=Axis.X,
                                    op=Alu.max)

            # ---- Phase B: one-shot all-to-all max gather.  Emitted
            # before the weight loop so the entry snapshot covers only
            # the x/amax work; wait_critical_data_deps lets the Pool
            # engine generate the descriptors during the amax and defers
            # the data gate to the trigger. ----
            inbox = small.tile([P, N_CORES], f32, name="inbox")
            gv = small.tile([P, 1], f32, name="gv")
            rsem = nc.alloc_semaphore("xmax_rsem")
            with tc.tile_critical(name="xmax"):
                me = nc.gpsimd.partition_id()
                rd = [(0, k) for k in range(N_CORES)]
                nc.gpsimd.remote_dma_broadcast(
                    inbox[:, DynSlice(me, 1)], cur0[:], rsem, lsem,
                    rdests=rd).then_inc(psem, 1)
                tc.wait_critical_data_deps()
                nc.gpsimd.wait_ge(psem, 2)
                nc.gpsimd.trigger_dma(count=1)
                nc.vector.wait_ge(rsem, 2 * N_CORES)
                nc.vector.tensor_reduce(out=gv[:], in_=inbox[:], axis=Axis.X,
                                        op=Alu.max)

            # identity + ones rows (cheap, consumed by the folds below)
            ident = small.tile([P, P], f32)
            make_identity(nc, ident[:])
            ones_row = small.tile([1, P], f32)
            nc.vector.memset(ones_row[:], 1.0)
            ones16 = small.tile([1, P], f16)
            nc.vector.memset(ones16[:], 1.0)

            # ---- scalar global max + scales (PE transpose fold, PE
            # ones-outer-product broadcast) ----
            lmax_t = psum.tile([1, P], f32, tag="ps", name="lmaxt")
            nc.tensor.transpose(lmax_t[:], gv[:], ident[:])
            gmax0 = small.tile([1, 1], f32)
            nc.vector.tensor_reduce(out=gmax0[:], in_=lmax_t[:], axis=Axis.X,
                                    op=Alu.max)
            # scale math on partition 0: col0 = inv_half, col1 = out_scale
            sc = small.tile([1, 2], f32)
            nc.vector.reciprocal(sc[:, 0:1], gmax0[:])
            nc.vector.tensor_scalar_mul(sc[:, 0:1], sc[:, 0:1], 224.0)
            nc.vector.tensor_scalar_mul(sc[:, 1:2], gmax0[:], 1.0 / 224.0)
            scps = psum.tile([P, 2], f32, tag="ps", name="scps")
            nc.tensor.matmul(scps[:], ones_row[:], sc[:], start=True,
                             stop=True)
            scales = small.tile([P, 2], f32)
            nc.vector.tensor_copy(scales[:], scps[:])
            inv_half = scales[:, 0:1]
            out_scale = scales[:, 1:2]

            # ---- weights: ring-ordered behind x; ACT casts j0-4, DVE
            # casts j5-7 (DVE is free once the scale chain is done) ----
            qw = []
            wstages = []
            for j in range(KP):
                stage = wstage.tile([P, 2, D_OUT], f16, tag="wst")
                src = wt[2 * j * P:(2 * j + 2) * P, :]
                dma = nc.sync.dma_start(stage[:],
                                        src.rearrange("(p t) n -> p t n", t=2))
                prev_dma = order(dma, prev_dma, "w-after-x")
                qt = qwpool.tile([P, 2, D_OUT], f8, tag="qw")
                if j < 5:
                    nc.scalar.activation(qt[:], stage[:], Act.Copy)
                qw.append(qt)
                wstages.append(stage)
            for j in (5, 6, 7):
                nc.vector.tensor_scalar(out=qw[j][:], in0=wstages[j][:],
                                        scalar1=1.0, scalar2=None,
                                        op0=Alu.mult)

            # ---- bias broadcast via PE ones-outer-product; PSUM->SBUF
            # copies on ACT (idle early, keeps DVE clear) ----
            bias_row = small.tile([1, D_OUT], f16)
            nc.scalar.dma_start(bias_row[:], bias[None, :])
            bias_bc = small.tile([P, D_OUT], f16)
            for nt in range(NT):
                bps = psum.tile([P, N_TILE], f32, tag="ps", name=f"bps{nt}")
                nc.tensor.matmul(bps[:], ones16[:],
                                 bias_row[:, nt * N_TILE:(nt + 1) * N_TILE],
                                 start=True, stop=True)
                nc.scalar.activation(
                    bias_bc[:, nt * N_TILE:(nt + 1) * N_TILE], bps[:],
                    Act.Copy)

            # trailing dummy collective for synchronized launch; the
            # cc_in DMA depends on scales so it cannot float early.
            nc.scalar.dma_start(cc_in[0:2], scales[0:1, :])
            nc.gpsimd.collective_compute(
                "AllGather", Alu.bypass, replica_groups=groups,
                ins=[cc_in.ap().opt()], outs=[cc_out.ap().opt()])

            # ---- Phases C+D interleaved ----
            qx = [qxpool.tile([P, 2, TOK_PC], f8, tag="qx", name=f"qx{j}")
                  for j in range(KP)]

            def quant(j, mt, eng):
                sl = slice(mt * P, (mt + 1) * P)
                if eng == "v":
                    nc.vector.tensor_scalar(out=qx[j][:, :, sl],
                                            in0=x_sb[j][:, :, sl],
                                            scalar1=inv_half[:, 0:1],
                                            scalar2=None, op0=Alu.mult)
                else:
                    nc.scalar.activation(qx[j][:, :, sl], x_sb[j][:, :, sl],
                                         Act.Copy, scale=inv_half[:, 0:1])

            def evict(mt, nt, ps):
                ysb = ypool.tile([P, N_TILE], f16, tag="ysb", name="ysb")
                nc.vector.scalar_tensor_tensor(
                    out=ysb[:], in0=ps[:], scalar=out_scale[:, 0:1],
                    in1=bias_bc[:, nt * N_TILE:(nt + 1) * N_TILE],
                    op0=Alu.mult, op1=Alu.add)
                nc.sync.dma_start(
                    y[mt * P:(mt + 1) * P, nt * N_TILE:(nt + 1) * N_TILE],
                    ysb[:])

            def mm(ps, mt, nt, j):
                nc.tensor.matmul(
                    ps[:],
                    qx[j][:, :, mt * P:(mt + 1) * P],
                    qw[j][:, :, nt * N_TILE:(nt + 1) * N_TILE],
                    start=(j == 0), stop=(j == KP - 1),
                    perf_mode=mybir.MatmulPerfMode.DoubleRow)

            # pair0 (mt0,mt1): quant j-major all-DVE, matmul j-outer
            for j in range(KP):
                for mt in (0, 1):
                    quant(j, mt, "v")
            pss = {(mt, nt): psum.tile([P, N_TILE], f32, tag="ps",
                                       name=f"p{mt}_{nt}")
                   for mt in (0, 1) for nt in range(NT)}
            for j in range(KP):
                for mt in (0, 1):
                    for nt in range(NT):
                        mm(pss[(mt, nt)], mt, nt, j)
            for mt in (0, 1):
                for nt in range(NT):
                    evict(mt, nt, pss[(mt, nt)])

            # mt2,mt3 quant fully on ACT (DVE handles pair0 evictions)
            for j in range(KP):
                for mt in (2, 3):
                    quant(j, mt, "a")
            for mt in range(2, MT):
                for nt in range(NT):
                    ps = psum.tile([P, N_TILE], f32, tag="ps", name=f"ps{nt}")
                    for j in range(KP):
                        mm(ps, mt, nt, j)
                    evict(mt, nt, ps)
                if mt + 2 < MT:
                    for j in range(KP):
                        quant(j, mt + 2, "v" if j % 2 == 0 else "a")

    nc.compile()
    return nc


def _get_compiled():
    global _compiled
    if _compiled is None:
        _compiled = _build()
    return _compiled


def run(x, weight, bias, **kw):
    """Shard + run on 8 cores; returns (full_output, BassKernelResults)."""
    from concourse.bass_utils import run_bass_kernel_spmd

    nc = _get_compiled()

    x = np.asarray(x, dtype=np.float16)
    weight = np.asarray(weight, dtype=np.float16)
    bias = np.asarray(bias, dtype=np.float16)
    xt = np.ascontiguousarray(x.reshape(TOK, D_IN).T)          # [d_in, tok]
    wt = np.ascontiguousarray(weight.T)                        # [d_in, d_out]
    in_maps = []
    for i in range(N_CORES):
        in_maps.append({
            "xt": np.ascontiguousarray(xt[:, i * TOK_PC:(i + 1) * TOK_PC]),
            "wt": wt,
            "bias": bias,
        })
    res = run_bass_kernel_spmd(nc, in_maps, core_ids=list(range(N_CORES)), **kw)
    out = np.concatenate([res.results[i]["y"] for i in range(N_CORES)], axis=0)
    return out.reshape(B, S, D_OUT), res


def kernel(x, weight, bias):
    out, _ = run(x, weight, bias)
    return out
